# revision 1
# baseline (speedup 1.0000x reference)
"""Trainium2 Bass kernel for batched masked attention (Z=8, S=2048, D=1024).

Strategy: pure data-parallel over batch z — each of the 8 NeuronCores computes
full attention for one batch element. No collectives.

Per-core dataflow (all matmuls, no on-chip transposes):
  - host passes xT = x[z].T  [D, S] and pre-transposed weights W.T [D, D] (bf16)
  - qT[a,s], kT[a,s]  = W.T-tile.T @ xT        (PE, contraction over input dim)
  - v[s,a]            = xT-tile.T @ Wv.T + bv  (bias folded in as a K=1 matmul)
  - zT[k,q]           = kT-tile.T @ qT         (scores with keys on partitions)
  - ET                = exp(zT/32 + kbias[k])  (ScalarE; key-mask folded into the
                                                per-partition bias: -30000 -> 0)
  - out_psum[q,a]     = ET-tile.T @ v          (PE, contraction over keys)
  - denom[q]          = ET-tile.T @ ones       (same stationary, N=1 matmul)
  - out[q,a]          = out_psum * (qscale[q] / denom[q])   (DVE, per-partition)

Query-masked rows are zeroed via qscale; softmax max-subtraction is unnecessary
(logits ~ N(0,1), exp is safe in f32) so the division can be deferred to the
output, which keeps everything in matmul-friendly layouts.
"""

import numpy as np
import ml_dtypes

P = 128
S = 2048  # sequence length
D = 1024  # model dim (= dim_qk = dim_v)
NI = D // P  # 8 contraction tiles for projections
NA = D // P  # 8 a-tiles
NS = S // P  # 16 s-tiles (key tiles)
QC = 512  # q-chunk width for the attention phase
NQC = S // QC  # 4 q-chunks
VC = 512  # v free-dim chunk
NVC = D // VC  # 2
SCALE = 1.0 / 32.0  # 1/sqrt(D)

_CACHE = {}


def _build_nc():
    from contextlib import ExitStack

    import concourse.tile as tile
    from concourse import bacc, mybir
    from concourse.bass import ts, ds

    f32 = mybir.dt.float32
    bf16 = mybir.dt.bfloat16
    EXP = mybir.ActivationFunctionType.Exp

    nc = bacc.Bacc(None, target_bir_lowering=False, debug=False)

    xt_d = nc.declare_dram_parameter("xt", [D, S], bf16, isOutput=False)
    wq_d = nc.declare_dram_parameter("wq", [D, D], bf16, isOutput=False)
    wk_d = nc.declare_dram_parameter("wk", [D, D], bf16, isOutput=False)
    wv_d = nc.declare_dram_parameter("wv", [D, D], bf16, isOutput=False)
    bv_d = nc.declare_dram_parameter("bv", [1, D], bf16, isOutput=False)
    kb_d = nc.declare_dram_parameter("kbias", [P, NS], f32, isOutput=False)
    qs_d = nc.declare_dram_parameter("qscale", [P, NS], f32, isOutput=False)
    out_d = nc.declare_dram_parameter("out", [S, D], f32, isOutput=True)

    with tile.TileContext(nc) as tc, ExitStack() as st:
        const = st.enter_context(tc.tile_pool(name="const", bufs=1))
        persist = st.enter_context(tc.tile_pool(name="persist", bufs=1))

        ones_col = const.tile([P, 1], bf16, name="ones_col", tag="ones_col")
        nc.vector.memset(ones_col, 1.0)
        ones_row = const.tile([1, P], bf16, name="ones_row", tag="ones_row")
        nc.vector.memset(ones_row, 1.0)
        kb_sb = const.tile([P, NS], f32, name="kb_sb", tag="kb_sb")
        nc.sync.dma_start(kb_sb, kb_d[:, :])
        qs_sb = const.tile([P, NS], f32, name="qs_sb", tag="qs_sb")
        nc.sync.dma_start(qs_sb, qs_d[:, :])
        bv_sb = const.tile([1, D], bf16, name="bv_sb", tag="bv_sb")
        nc.sync.dma_start(bv_sb, bv_d[:, :])

        qt = [
            persist.tile([P, S], bf16, name=f"qt{a}", tag="qt", bufs=NA)
            for a in range(NA)
        ]
        kt = [
            persist.tile([P, S], bf16, name=f"kt{a}", tag="kt", bufs=NA)
            for a in range(NA)
        ]
        v = [
            persist.tile([P, D], bf16, name=f"v{s}", tag="v", bufs=NS)
            for s in range(NS)
        ]

        # ---- phase 1: projections -------------------------------------
        with (
            tc.tile_pool(name="xw", bufs=1) as xw,
            tc.tile_pool(name="proj_psum", bufs=8, space="PSUM") as pp,
        ):
            xts = []
            for it in range(NI):
                t = xw.tile([P, S], bf16, name=f"xtile{it}", tag="xt", bufs=NI)
                nc.sync.dma_start(t, xt_d[ts(it, P), :])
                xts.append(t)

            def load_w(dram, label):
                tiles = []
                for it in range(NI):
                    t = xw.tile([P, D], bf16, name=f"{label}{it}", tag="w", bufs=16)
                    nc.sync.dma_start(t, dram[ts(it, P), :])
                    tiles.append(t)
                return tiles

            wq_t = load_w(wq_d, "wqt")
            wk_t = load_w(wk_d, "wkt")
            wv_t = load_w(wv_d, "wvt")

            # qT / kT: out[a-tile, s-chunk] = sum_i W.T[i, a-tile].T @ xT[i, s-chunk]
            for wt, dst, lbl in ((wq_t, qt, "q"), (wk_t, kt, "k")):
                for a in range(NA):
                    pss = [
                        pp.tile([P, QC], f32, name=f"pp_{lbl}{a}_{c}", tag="pp")
                        for c in range(NQC)
                    ]
                    for it in range(NI):
                        for c in range(NQC):
                            nc.tensor.matmul(
                                pss[c],
                                lhsT=wt[it][:, ts(a, P)],
                                rhs=xts[it][:, ts(c, QC)],
                                start=(it == 0),
                                stop=(it == NI - 1),
                            )
                    for c in range(NQC):
                        nc.vector.tensor_copy(dst[a][:, ts(c, QC)], pss[c])

            # v: out[s-tile, a-chunk] = sum_i xT[i, s-tile].T @ Wv.T[i, a-chunk] + bv
            for s16 in range(NS):
                pss = [
                    pp.tile([P, VC], f32, name=f"pp_v{s16}_{c}", tag="pp")
                    for c in range(NVC)
                ]
                for it in range(NI):
                    for c in range(NVC):
                        nc.tensor.matmul(
                            pss[c],
                            lhsT=xts[it][:, ts(s16, P)],
                            rhs=wv_t[it][:, ts(c, VC)],
                            start=(it == 0),
                            stop=False,
                        )
                for c in range(NVC):
                    nc.tensor.matmul(
                        pss[c],
                        lhsT=ones_row[:1, :],
                        rhs=bv_sb[:1, ts(c, VC)],
                        start=False,
                        stop=True,
                    )
                    nc.vector.tensor_copy(v[s16][:, ts(c, VC)], pss[c])

        # ---- phase 2: attention ---------------------------------------
        with (
            tc.tile_pool(name="etp", bufs=1) as etp,
            tc.tile_pool(name="zp", bufs=2, space="PSUM") as zp,
            tc.tile_pool(name="pvp", bufs=4, space="PSUM") as pvp,
            tc.tile_pool(name="dnp", bufs=2, space="PSUM") as dnp,
            tc.tile_pool(name="outp", bufs=4) as outp,
            tc.tile_pool(name="smol", bufs=8) as smol,
        ):
            for qc in range(NQC):
                ets = []
                for k16 in range(NS):
                    zps = zp.tile([P, QC], f32, name=f"z{qc}_{k16}", tag="z")
                    for a in range(NA):
                        nc.tensor.matmul(
                            zps,
                            lhsT=kt[a][:, ts(k16, P)],
                            rhs=qt[a][:, ts(qc, QC)],
                            start=(a == 0),
                            stop=(a == NA - 1),
                        )
                    et = etp.tile(
                        [P, QC], bf16, name=f"et{qc}_{k16}", tag="et", bufs=2 * NS
                    )
                    nc.scalar.activation(
                        et, zps, EXP, bias=kb_sb[:, k16 : k16 + 1], scale=SCALE
                    )
                    ets.append(et)

                for q4 in range(QC // P):
                    qtile = qc * (QC // P) + q4  # global q-tile index
                    opss = [
                        pvp.tile([P, VC], f32, name=f"pv{qtile}_{c}", tag="pv")
                        for c in range(NVC)
                    ]
                    dps = dnp.tile([P, 1], f32, name=f"dn{qtile}", tag="dn")
                    for k16 in range(NS):
                        lhs = ets[k16][:, ts(q4, P)]
                        for c in range(NVC):
                            nc.tensor.matmul(
                                opss[c],
                                lhsT=lhs,
                                rhs=v[k16][:, ts(c, VC)],
                                start=(k16 == 0),
                                stop=(k16 == NS - 1),
                            )
                        nc.tensor.matmul(
                            dps,
                            lhsT=lhs,
                            rhs=ones_col[:, :1],
                            start=(k16 == 0),
                            stop=(k16 == NS - 1),
                        )
                    rec = smol.tile([P, 1], f32, name=f"rec{qtile}", tag="rec")
                    nc.vector.reciprocal(rec, dps)
                    scl = smol.tile([P, 1], f32, name=f"scl{qtile}", tag="scl")
                    nc.vector.tensor_mul(scl, rec, qs_sb[:, qtile : qtile + 1])
                    for c in range(NVC):
                        ot = outp.tile([P, VC], f32, name=f"ot{qtile}_{c}", tag="ot")
                        nc.vector.tensor_scalar_mul(ot, opss[c], scl)
                        nc.sync.dma_start(out_d[ds(qtile * P, P), ts(c, VC)], ot)

    nc.compile()
    return nc


def _get_nc():
    if "nc" not in _CACHE:
        _CACHE["nc"] = _build_nc()
    return _CACHE["nc"]


def _make_in_maps(x, Wq, Wk, Wv, bv, mask):
    bf16 = ml_dtypes.bfloat16
    wq_t = np.ascontiguousarray(Wq.astype(np.float32).T).astype(bf16)
    wk_t = np.ascontiguousarray(Wk.astype(np.float32).T).astype(bf16)
    wv_t = np.ascontiguousarray(Wv.astype(np.float32).T).astype(bf16)
    bv_row = bv.astype(np.float32).reshape(1, D).astype(bf16)
    in_maps = []
    for z in range(8):
        xT = np.ascontiguousarray(x[z].astype(np.float32).T).astype(bf16)
        mz = mask[z].astype(bool)
        kb = np.where(mz, np.float32(-30000.0), np.float32(0.0))
        qs = np.where(mz, np.float32(0.0), np.float32(1.0))
        # SBUF layout [p, t]: element (p, t) = vec[t*128 + p]
        kb = np.ascontiguousarray(kb.reshape(NS, P).T)
        qs = np.ascontiguousarray(qs.reshape(NS, P).T)
        in_maps.append(
            {
                "xt": xT,
                "wq": wq_t,
                "wk": wk_t,
                "wv": wv_t,
                "bv": bv_row,
                "kbias": kb,
                "qscale": qs,
            }
        )
    return in_maps


def run(x, Wq, Wk, Wv, bv, mask, trace=False):
    from concourse.bass_utils import run_bass_kernel_spmd

    nc = _get_nc()
    in_maps = _make_in_maps(x, Wq, Wk, Wv, bv, mask)
    res = run_bass_kernel_spmd(nc, in_maps, core_ids=list(range(8)), trace=trace)
    out = np.stack([res.results[z]["out"] for z in range(8)]).astype(np.float32)
    return out, res


def kernel(x, Wq, Wk, Wv, bv, mask):
    out, _ = run(x, Wq, Wk, Wv, bv, mask, trace=False)
    return out


# revision 2
# speedup vs baseline: 2.2345x; 2.2345x over previous
"""Trainium2 Bass kernel for batched masked attention (Z=8, S=2048, D=1024).

Strategy: pure data-parallel over batch z — each of the 8 NeuronCores computes
full attention for one batch element. No collectives.

Mask compaction: the reference's symmetric mask kills row q and column k
whenever position is masked (masked-query rows are exactly 0 in the output,
masked-key columns contribute exactly 0 to every sum). Query-mask == key-mask,
so the host gathers only the unmasked positions (~half), padded to a multiple
of 128 shared across cores, runs dense attention on the compacted sequence,
and scatters the result rows back into a zero output. Bit-equivalent math at
~40% of the dense FLOPs.

Per-core dataflow (all matmuls, no on-chip transposes):
  - host passes xcT = x[z][idx].T  [D, N] and pre-transposed weights W.T (bf16)
  - qT[a,s], kT[a,s]  = W.T-tile.T @ xcT       (PE, contraction over input dim)
  - v[s,a]            = xcT-tile.T @ Wv.T + bv (bias folded in as a K=1 matmul)
  - zT[k,q]           = kT-tile.T @ qT         (scores with keys on partitions)
  - ET                = exp(zT/32 + kbias[k])  (ScalarE; padding keys get
                                                bias -30000 -> exp underflows to 0)
  - out_psum[q,a]     = ET-tile.T @ v          (PE, contraction over keys)
  - denom[q]          = ET-tile.T @ ones       (same stationary, N=1 matmul)
  - out[q,a]          = out_psum / denom[q]    (DVE, per-partition scale)

No softmax max-subtraction is needed (logits ~ N(0,1); exp is safe in f32),
which is what lets the division defer to the output and keeps every stage in
a matmul-friendly layout.
"""

import numpy as np
import ml_dtypes

P = 128
S = 2048  # full sequence length
D = 1024  # model dim (= dim_qk = dim_v)
NI = D // P  # 8 contraction tiles for projections
NA = D // P  # 8 a-tiles
VC = 512  # v free-dim chunk
NVC = D // VC  # 2
SCALE = 1.0 / 32.0  # 1/sqrt(D)

_CACHE = {}


def _chunks(total, maxw):
    out = []
    off = 0
    while off < total:
        w = min(maxw, total - off)
        out.append((off, w))
        off += w
    return out


def _build_nc(nk):
    """Build the per-core graph for a compacted sequence of N = nk*128."""
    from contextlib import ExitStack

    import concourse.tile as tile
    from concourse import bacc, mybir
    from concourse.bass import ts, ds

    N = nk * P
    f32 = mybir.dt.float32
    bf16 = mybir.dt.bfloat16
    EXP = mybir.ActivationFunctionType.Exp

    nc = bacc.Bacc(None, target_bir_lowering=False, debug=False)

    xc_d = nc.declare_dram_parameter("xc", [D, N], bf16, isOutput=False)
    wq_d = nc.declare_dram_parameter("wq", [D, D], bf16, isOutput=False)
    wk_d = nc.declare_dram_parameter("wk", [D, D], bf16, isOutput=False)
    wv_d = nc.declare_dram_parameter("wv", [D, D], bf16, isOutput=False)
    bv_d = nc.declare_dram_parameter("bv", [1, D], bf16, isOutput=False)
    kb_d = nc.declare_dram_parameter("kbias", [P, nk], f32, isOutput=False)
    out_d = nc.declare_dram_parameter("out", [N, D], f32, isOutput=True)

    qchunks = _chunks(N, 512)

    with tile.TileContext(nc) as tc, ExitStack() as st:
        const = st.enter_context(tc.tile_pool(name="const", bufs=1))
        persist = st.enter_context(tc.tile_pool(name="persist", bufs=1))

        ones_col = const.tile([P, 1], bf16, name="ones_col", tag="ones_col")
        nc.vector.memset(ones_col, 1.0)
        ones_row = const.tile([1, P], bf16, name="ones_row", tag="ones_row")
        nc.vector.memset(ones_row, 1.0)
        kb_sb = const.tile([P, nk], f32, name="kb_sb", tag="kb_sb")
        nc.sync.dma_start(kb_sb, kb_d[:, :])
        bv_sb = const.tile([1, D], bf16, name="bv_sb", tag="bv_sb")
        nc.sync.dma_start(bv_sb, bv_d[:, :])

        qt = [
            persist.tile([P, N], bf16, name=f"qt{a}", tag="qt", bufs=NA)
            for a in range(NA)
        ]
        kt = [
            persist.tile([P, N], bf16, name=f"kt{a}", tag="kt", bufs=NA)
            for a in range(NA)
        ]
        v = [
            persist.tile([P, D], bf16, name=f"v{s}", tag="v", bufs=nk)
            for s in range(nk)
        ]

        # ---- phase 1: projections -------------------------------------
        with (
            tc.tile_pool(name="xw", bufs=1) as xw,
            tc.tile_pool(name="proj_psum", bufs=8, space="PSUM") as pp,
        ):
            # interleave xc / wq loads so the first q-projection group can
            # start as soon as the first pair lands (cuts the PE lead-in)
            xts, wq_t = [], []
            for it in range(NI):
                t = xw.tile([P, N], bf16, name=f"xtile{it}", tag="xt", bufs=NI)
                nc.sync.dma_start(t, xc_d[ts(it, P), :])
                xts.append(t)
                w = xw.tile([P, D], bf16, name=f"wqt{it}", tag="w", bufs=24)
                nc.sync.dma_start(w, wq_d[ts(it, P), :])
                wq_t.append(w)

            def load_w(dram, label):
                tiles = []
                for it in range(NI):
                    t = xw.tile([P, D], bf16, name=f"{label}{it}", tag="w", bufs=24)
                    nc.sync.dma_start(t, dram[ts(it, P), :])
                    tiles.append(t)
                return tiles

            wk_t = load_w(wk_d, "wkt")
            wv_t = load_w(wv_d, "wvt")

            # qT / kT: out[a-tile, s-chunk] = sum_i W.T[i, a-tile].T @ xcT[i, chunk]
            for wt, dst, lbl in ((wq_t, qt, "q"), (wk_t, kt, "k")):
                for a in range(NA):
                    pss = [
                        pp.tile([P, w], f32, name=f"pp_{lbl}{a}_{ci}", tag="pp")
                        for ci, (off, w) in enumerate(qchunks)
                    ]
                    for it in range(NI):
                        for ci, (off, w) in enumerate(qchunks):
                            nc.tensor.matmul(
                                pss[ci],
                                lhsT=wt[it][:, ts(a, P)],
                                rhs=xts[it][:, ds(off, w)],
                                start=(it == 0),
                                stop=(it == NI - 1),
                            )
                    for ci, (off, w) in enumerate(qchunks):
                        nc.vector.tensor_copy(dst[a][:, ds(off, w)], pss[ci])

            # v: out[s-tile, a-chunk] = sum_i xcT[i, s-tile].T @ Wv.T[i, chunk] + bv
            for s16 in range(nk):
                pss = [
                    pp.tile([P, VC], f32, name=f"pp_v{s16}_{c}", tag="pp")
                    for c in range(NVC)
                ]
                for it in range(NI):
                    for c in range(NVC):
                        nc.tensor.matmul(
                            pss[c],
                            lhsT=xts[it][:, ts(s16, P)],
                            rhs=wv_t[it][:, ts(c, VC)],
                            start=(it == 0),
                            stop=False,
                        )
                for c in range(NVC):
                    nc.tensor.matmul(
                        pss[c],
                        lhsT=ones_row[:1, :],
                        rhs=bv_sb[:1, ts(c, VC)],
                        start=False,
                        stop=True,
                    )
                    nc.vector.tensor_copy(v[s16][:, ts(c, VC)], pss[c])

        # ---- phase 2: attention ---------------------------------------
        with (
            tc.tile_pool(name="etp", bufs=1) as etp,
            tc.tile_pool(name="zp", bufs=2, space="PSUM") as zp,
            tc.tile_pool(name="pvp", bufs=4, space="PSUM") as pvp,
            tc.tile_pool(name="dnp", bufs=2, space="PSUM") as dnp,
            tc.tile_pool(name="outp", bufs=4) as outp,
            tc.tile_pool(name="smol", bufs=8) as smol,
        ):
            for qc, (qoff, qw) in enumerate(qchunks):
                ets = []
                for k16 in range(nk):
                    zps = zp.tile([P, qw], f32, name=f"z{qc}_{k16}", tag="z")
                    for a in range(NA):
                        nc.tensor.matmul(
                            zps,
                            lhsT=kt[a][:, ts(k16, P)],
                            rhs=qt[a][:, ds(qoff, qw)],
                            start=(a == 0),
                            stop=(a == NA - 1),
                        )
                    et = etp.tile(
                        [P, qw],
                        bf16,
                        name=f"et{qc}_{k16}",
                        tag="et",
                        bufs=2 * nk,
                        padded_shape=[P, 512],
                    )
                    nc.scalar.activation(
                        et, zps, EXP, bias=kb_sb[:, k16 : k16 + 1], scale=SCALE
                    )
                    ets.append(et)

                for q4 in range(qw // P):
                    qtile = qoff // P + q4  # global q-tile index
                    opss = [
                        pvp.tile([P, VC], f32, name=f"pv{qtile}_{c}", tag="pv")
                        for c in range(NVC)
                    ]
                    dps = dnp.tile([P, 1], f32, name=f"dn{qtile}", tag="dn")
                    for k16 in range(nk):
                        lhs = ets[k16][:, ts(q4, P)]
                        for c in range(NVC):
                            nc.tensor.matmul(
                                opss[c],
                                lhsT=lhs,
                                rhs=v[k16][:, ts(c, VC)],
                                start=(k16 == 0),
                                stop=(k16 == nk - 1),
                            )
                        nc.tensor.matmul(
                            dps,
                            lhsT=lhs,
                            rhs=ones_col[:, :1],
                            start=(k16 == 0),
                            stop=(k16 == nk - 1),
                        )
                    rec = smol.tile([P, 1], f32, name=f"rec{qtile}", tag="rec")
                    nc.vector.reciprocal(rec, dps)
                    for c in range(NVC):
                        ot = outp.tile([P, VC], f32, name=f"ot{qtile}_{c}", tag="ot")
                        nc.vector.tensor_scalar_mul(ot, opss[c], rec)
                        nc.sync.dma_start(out_d[ds(qtile * P, P), ts(c, VC)], ot)

    nc.compile()
    return nc


def _get_nc(nk):
    if nk not in _CACHE:
        _CACHE[nk] = _build_nc(nk)
    return _CACHE[nk]


def _make_in_maps(x, Wq, Wk, Wv, bv, mask, idxs, nk):
    bf16 = ml_dtypes.bfloat16
    N = nk * P
    wq_t = np.ascontiguousarray(Wq.astype(np.float32).T).astype(bf16)
    wk_t = np.ascontiguousarray(Wk.astype(np.float32).T).astype(bf16)
    wv_t = np.ascontiguousarray(Wv.astype(np.float32).T).astype(bf16)
    bv_row = bv.astype(np.float32).reshape(1, D).astype(bf16)
    in_maps = []
    for z in range(8):
        idx = idxs[z]
        n = idx.size
        idx_pad = np.zeros(N, dtype=np.int64)
        idx_pad[:n] = idx
        xc = np.ascontiguousarray(x[z][idx_pad].astype(np.float32).T).astype(bf16)
        kb = np.full(N, -30000.0, dtype=np.float32)
        kb[:n] = 0.0
        # SBUF layout [p, t]: element (p, t) = vec[t*128 + p]
        kb = np.ascontiguousarray(kb.reshape(nk, P).T)
        in_maps.append(
            {
                "xc": xc,
                "wq": wq_t,
                "wk": wk_t,
                "wv": wv_t,
                "bv": bv_row,
                "kbias": kb,
            }
        )
    return in_maps


def run(x, Wq, Wk, Wv, bv, mask, trace=False):
    from concourse.bass_utils import run_bass_kernel_spmd

    x = np.asarray(x)
    mask = np.asarray(mask).astype(bool)
    idxs = [np.nonzero(~mask[z])[0] for z in range(8)]
    nmax = max(int(i.size) for i in idxs)
    nk = max(1, -(-nmax // P))  # ceil to tile count shared by all cores
    nc = _get_nc(nk)
    in_maps = _make_in_maps(x, Wq, Wk, Wv, bv, mask, idxs, nk)
    res = run_bass_kernel_spmd(nc, in_maps, core_ids=list(range(8)), trace=trace)
    out = np.zeros((8, S, D), dtype=np.float32)
    for z in range(8):
        n = idxs[z].size
        if n:
            out[z][idxs[z]] = res.results[z]["out"][:n].astype(np.float32)
    return out, res


def kernel(x, Wq, Wk, Wv, bv, mask):
    out, _ = run(x, Wq, Wk, Wv, bv, mask, trace=False)
    return out


# revision 6
# speedup vs baseline: 2.3254x; 1.0407x over previous
"""Trainium2 Bass kernel for batched masked attention (Z=8, S=2048, D=1024).

Strategy: pure data-parallel over batch z — each of the 8 NeuronCores computes
full attention for one batch element. No collectives.

Mask compaction: the reference's symmetric mask kills row q and column k
whenever position is masked (masked-query rows are exactly 0 in the output,
masked-key columns contribute exactly 0 to every sum). Query-mask == key-mask,
so the host gathers only the unmasked positions (~half), padded to a multiple
of 128 shared across cores, runs dense attention on the compacted sequence,
and scatters the result rows back into a zero output. Bit-equivalent math at
~40% of the dense FLOPs.

Per-core dataflow (all matmuls, no on-chip transposes):
  - host passes xcT = x[z][idx].T  [D, N] and pre-transposed weights W.T (bf16)
  - qT[a,s], kT[a,s]  = W.T-tile.T @ xcT       (PE, contraction over input dim)
  - v[s,a]            = xcT-tile.T @ Wv.T + bv (bias folded in as a K=1 matmul)
  - zT[k,q]           = kT-tile.T @ qT         (scores with keys on partitions)
  - ET                = exp(zT/32 + kbias[k])  (ScalarE; padding keys get
                                                bias -30000 -> exp underflows to 0)
  - out_psum[q,a]     = ET-tile.T @ v          (PE, contraction over keys)
  - denom[q]          = ET-tile.T @ ones       (same stationary, N=1 matmul)
  - out[q,a]          = out_psum / denom[q]    (DVE, per-partition scale)

No softmax max-subtraction is needed (logits ~ N(0,1); exp is safe in f32),
which is what lets the division defer to the output and keeps every stage in
a matmul-friendly layout.
"""

import numpy as np
import ml_dtypes

P = 128
S = 2048  # full sequence length
D = 1024  # model dim (= dim_qk = dim_v)
NI = D // P  # 8 contraction tiles for projections
NA = D // P  # 8 a-tiles
VC = 512  # v free-dim chunk
NVC = D // VC  # 2
SCALE = 1.0 / 32.0  # 1/sqrt(D)

_CACHE = {}


def _chunks(total, maxw):
    out = []
    off = 0
    while off < total:
        w = min(maxw, total - off)
        out.append((off, w))
        off += w
    return out


def _build_nc(nk):
    """Build the per-core graph for a compacted sequence of N = nk*128."""
    from contextlib import ExitStack

    import concourse.tile as tile
    from concourse import bacc, mybir
    from concourse.bass import ts, ds

    N = nk * P
    f32 = mybir.dt.float32
    bf16 = mybir.dt.bfloat16
    EXP = mybir.ActivationFunctionType.Exp

    nc = bacc.Bacc(None, target_bir_lowering=False, debug=False)

    xc_d = nc.declare_dram_parameter("xc", [D, N], bf16, isOutput=False)
    wq_d = nc.declare_dram_parameter("wq", [D, D], bf16, isOutput=False)
    wk_d = nc.declare_dram_parameter("wk", [D, D], bf16, isOutput=False)
    wv_d = nc.declare_dram_parameter("wv", [D, D], bf16, isOutput=False)
    bv_d = nc.declare_dram_parameter("bv", [1, D], f32, isOutput=False)
    kb_d = nc.declare_dram_parameter("kbias", [P, nk], f32, isOutput=False)
    out_d = nc.declare_dram_parameter("out", [N, D], f32, isOutput=True)

    qchunks = _chunks(N, 512)

    with tile.TileContext(nc) as tc, ExitStack() as st:
        const = st.enter_context(tc.tile_pool(name="const", bufs=1))
        persist = st.enter_context(tc.tile_pool(name="persist", bufs=1))

        ones_col = const.tile([P, 1], bf16, name="ones_col", tag="ones_col")
        nc.gpsimd.memset(ones_col, 1.0)
        kb_sb = const.tile([P, nk], f32, name="kb_sb", tag="kb_sb")
        nc.sync.dma_start(kb_sb, kb_d[:, :])
        bv_sb = const.tile([1, D], f32, name="bv_sb", tag="bv_sb")
        nc.sync.dma_start(bv_sb, bv_d[:, :])
        bv_bc = const.tile([P, D], f32, name="bv_bc", tag="bv_bc")
        nc.gpsimd.partition_broadcast(bv_bc, bv_sb[:1, :])

        # PE pre-warm: dummy matmuls with no data deps run during the input
        # DMA lead-in so HAM un-throttles before the first real matmul.
        ws = const.tile([P, P], bf16, name="ws", tag="ws")
        nc.gpsimd.memset(ws, 0.0)
        with tc.tile_pool(name="warm_psum", bufs=1, space="PSUM") as wpp:
            wp = wpp.tile([P, P], f32, name="wp", tag="wp")
            for _ in range(72):
                nc.tensor.matmul(wp, lhsT=ws, rhs=ws, start=True, stop=True)

        qt = [
            persist.tile([P, N], bf16, name=f"qt{a}", tag="qt", bufs=NA)
            for a in range(NA)
        ]
        kt = [
            persist.tile([P, N], bf16, name=f"kt{a}", tag="kt", bufs=NA)
            for a in range(NA)
        ]
        v = [
            persist.tile([P, D], bf16, name=f"v{s}", tag="v", bufs=nk)
            for s in range(nk)
        ]

        # ---- phase 1: projections -------------------------------------
        with (
            tc.tile_pool(name="xw", bufs=1) as xw,
            tc.tile_pool(name="proj_psum", bufs=8, space="PSUM") as pp,
        ):
            # interleave xc / wq loads so the first q-projection group can
            # start as soon as the first pair lands (cuts the PE lead-in)
            xts, wq_t = [], []
            for it in range(NI):
                t = xw.tile([P, N], bf16, name=f"xtile{it}", tag="xt", bufs=NI)
                nc.sync.dma_start(t, xc_d[ts(it, P), :])
                xts.append(t)
                w = xw.tile([P, D], bf16, name=f"wqt{it}", tag="w", bufs=24)
                nc.sync.dma_start(w, wq_d[ts(it, P), :])
                wq_t.append(w)

            def load_w(dram, label):
                tiles = []
                for it in range(NI):
                    t = xw.tile([P, D], bf16, name=f"{label}{it}", tag="w", bufs=24)
                    nc.sync.dma_start(t, dram[ts(it, P), :])
                    tiles.append(t)
                return tiles

            wk_t = load_w(wk_d, "wkt")
            wv_t = load_w(wv_d, "wvt")

            # qT / kT: out[a-tile, s-chunk] = sum_i W.T[i, a-tile].T @ xcT[i, chunk]
            for wt, dst, lbl in ((wq_t, qt, "q"), (wk_t, kt, "k")):
                for a in range(NA):
                    pss = [
                        pp.tile([P, w], f32, name=f"pp_{lbl}{a}_{ci}", tag="pp")
                        for ci, (off, w) in enumerate(qchunks)
                    ]
                    for it in range(NI):
                        for ci, (off, w) in enumerate(qchunks):
                            nc.tensor.matmul(
                                pss[ci],
                                lhsT=wt[it][:, ts(a, P)],
                                rhs=xts[it][:, ds(off, w)],
                                start=(it == 0),
                                stop=(it == NI - 1),
                            )
                    for ci, (off, w) in enumerate(qchunks):
                        nc.vector.tensor_copy(dst[a][:, ds(off, w)], pss[ci])

            # v: out[s-tile, a-chunk] = sum_i xcT[i, s-tile].T @ Wv.T[i, chunk] + bv
            for s16 in range(nk):
                pss = [
                    pp.tile([P, VC], f32, name=f"pp_v{s16}_{c}", tag="pp")
                    for c in range(NVC)
                ]
                for it in range(NI):
                    for c in range(NVC):
                        nc.tensor.matmul(
                            pss[c],
                            lhsT=xts[it][:, ts(s16, P)],
                            rhs=wv_t[it][:, ts(c, VC)],
                            start=(it == 0),
                            stop=(it == NI - 1),
                        )
                for c in range(NVC):
                    nc.vector.tensor_add(
                        v[s16][:, ts(c, VC)], pss[c], bv_bc[:, ts(c, VC)]
                    )

        # ---- phase 2: attention ---------------------------------------
        with (
            tc.tile_pool(name="etp", bufs=1) as etp,
            tc.tile_pool(name="zp", bufs=2, space="PSUM") as zp,
            tc.tile_pool(name="pvp", bufs=4, space="PSUM") as pvp,
            tc.tile_pool(name="dnp", bufs=2, space="PSUM") as dnp,
            tc.tile_pool(name="outp", bufs=4) as outp,
            tc.tile_pool(name="smol", bufs=8) as smol,
        ):
            for qc, (qoff, qw) in enumerate(qchunks):
                ets = []
                for k16 in range(nk):
                    zps = zp.tile([P, qw], f32, name=f"z{qc}_{k16}", tag="z")
                    for a in range(NA):
                        nc.tensor.matmul(
                            zps,
                            lhsT=kt[a][:, ts(k16, P)],
                            rhs=qt[a][:, ds(qoff, qw)],
                            start=(a == 0),
                            stop=(a == NA - 1),
                        )
                    et = etp.tile(
                        [P, qw],
                        bf16,
                        name=f"et{qc}_{k16}",
                        tag="et",
                        bufs=2 * nk,
                        padded_shape=[P, 512],
                    )
                    nc.scalar.activation(
                        et, zps, EXP, bias=kb_sb[:, k16 : k16 + 1], scale=SCALE
                    )
                    ets.append(et)

                for q4 in range(qw // P):
                    qtile = qoff // P + q4  # global q-tile index
                    opss = [
                        pvp.tile([P, VC], f32, name=f"pv{qtile}_{c}", tag="pv")
                        for c in range(NVC)
                    ]
                    dps = dnp.tile([P, 1], f32, name=f"dn{qtile}", tag="dn")
                    for k16 in range(nk):
                        lhs = ets[k16][:, ts(q4, P)]
                        for c in range(NVC):
                            nc.tensor.matmul(
                                opss[c],
                                lhsT=lhs,
                                rhs=v[k16][:, ts(c, VC)],
                                start=(k16 == 0),
                                stop=(k16 == nk - 1),
                            )
                        nc.tensor.matmul(
                            dps,
                            lhsT=lhs,
                            rhs=ones_col[:, :1],
                            start=(k16 == 0),
                            stop=(k16 == nk - 1),
                        )
                    rec = smol.tile([P, 1], f32, name=f"rec{qtile}", tag="rec")
                    nc.vector.reciprocal(rec, dps)
                    for c in range(NVC):
                        ot = outp.tile([P, VC], f32, name=f"ot{qtile}_{c}", tag="ot")
                        nc.vector.tensor_scalar_mul(ot, opss[c], rec)
                        nc.sync.dma_start(out_d[ds(qtile * P, P), ts(c, VC)], ot)

    nc.compile()
    return nc


def _get_nc(nk):
    if nk not in _CACHE:
        _CACHE[nk] = _build_nc(nk)
    return _CACHE[nk]


def _make_in_maps(x, Wq, Wk, Wv, bv, mask, idxs, nk):
    bf16 = ml_dtypes.bfloat16
    N = nk * P
    wq_t = np.ascontiguousarray(Wq.astype(np.float32).T).astype(bf16)
    wk_t = np.ascontiguousarray(Wk.astype(np.float32).T).astype(bf16)
    wv_t = np.ascontiguousarray(Wv.astype(np.float32).T).astype(bf16)
    bv_row = np.ascontiguousarray(bv.astype(np.float32).reshape(1, D))
    in_maps = []
    for z in range(8):
        idx = idxs[z]
        n = idx.size
        idx_pad = np.zeros(N, dtype=np.int64)
        idx_pad[:n] = idx
        xc = np.ascontiguousarray(x[z][idx_pad].astype(np.float32).T).astype(bf16)
        kb = np.full(N, -30000.0, dtype=np.float32)
        kb[:n] = 0.0
        # SBUF layout [p, t]: element (p, t) = vec[t*128 + p]
        kb = np.ascontiguousarray(kb.reshape(nk, P).T)
        in_maps.append(
            {
                "xc": xc,
                "wq": wq_t,
                "wk": wk_t,
                "wv": wv_t,
                "bv": bv_row,
                "kbias": kb,
            }
        )
    return in_maps


def run(x, Wq, Wk, Wv, bv, mask, trace=False):
    from concourse.bass_utils import run_bass_kernel_spmd

    x = np.asarray(x)
    mask = np.asarray(mask).astype(bool)
    idxs = [np.nonzero(~mask[z])[0] for z in range(8)]
    nmax = max(int(i.size) for i in idxs)
    nk = max(1, -(-nmax // P))  # ceil to tile count shared by all cores
    nc = _get_nc(nk)
    in_maps = _make_in_maps(x, Wq, Wk, Wv, bv, mask, idxs, nk)
    res = run_bass_kernel_spmd(nc, in_maps, core_ids=list(range(8)), trace=trace)
    out = np.zeros((8, S, D), dtype=np.float32)
    for z in range(8):
        n = idxs[z].size
        if n:
            out[z][idxs[z]] = res.results[z]["out"][:n].astype(np.float32)
    return out, res


def kernel(x, Wq, Wk, Wv, bv, mask):
    out, _ = run(x, Wq, Wk, Wv, bv, mask, trace=False)
    return out


# revision 7
# speedup vs baseline: 2.3416x; 1.0070x over previous
"""Trainium2 Bass kernel for batched masked attention (Z=8, S=2048, D=1024).

Strategy: pure data-parallel over batch z — each of the 8 NeuronCores computes
full attention for one batch element. No collectives.

Mask compaction: the reference's symmetric mask kills row q and column k
whenever position is masked (masked-query rows are exactly 0 in the output,
masked-key columns contribute exactly 0 to every sum). Query-mask == key-mask,
so the host gathers only the unmasked positions (~half), padded to a multiple
of 64 shared across cores, runs dense attention on the compacted sequence,
and scatters the result rows back into a zero output. Bit-equivalent math at
~40% of the dense FLOPs.

Per-core dataflow (all matmuls, no on-chip transposes):
  - host passes xcT = x[z][idx].T  [D, N] and pre-transposed weights W.T (bf16)
  - qT[a,s], kT[a,s]  = W.T-tile.T @ xcT       (PE, contraction over input dim)
  - v[s,a]            = xcT-tile.T @ Wv.T + bv (bias added on DVE from a
                                                partition-broadcast bv row)
  - zT[k,q]           = kT-tile.T @ qT         (scores with keys on partitions)
  - ET                = exp(zT/32 + kbias[k])  (ScalarE; padding keys get
                                                bias -30000 -> exp underflows to 0)
  - out_psum[q,a]     = ET-tile.T @ v          (PE, contraction over keys)
  - denom[q]          = ET-tile.T @ ones       (same stationary, N=1 matmul)
  - out[q,a]          = out_psum / denom[q]    (DVE, per-partition scale)

No softmax max-subtraction is needed (logits ~ N(0,1); exp is safe in f32),
which is what lets the division defer to the output and keeps every stage in
a matmul-friendly layout. PE is pre-warmed with dummy matmuls during the input
DMA lead-in; output stores are split across partitions so the last store does
not serialize on a single DMA queue.
"""

import numpy as np
import ml_dtypes

P = 128
S = 2048  # full sequence length
D = 1024  # model dim (= dim_qk = dim_v)
NI = D // P  # 8 contraction tiles for projections
NA = D // P  # 8 a-tiles
VC = 512  # v free-dim chunk
NVC = D // VC  # 2
SCALE = 1.0 / 32.0  # 1/sqrt(D)
GRAN = 64  # sequence padding granularity
NWARM = 72  # PE pre-warm dummy matmuls
OSPLIT = 32  # output-store partition split

_CACHE = {}


def _chunks(total, maxw):
    out = []
    off = 0
    while off < total:
        w = min(maxw, total - off)
        out.append((off, w))
        off += w
    return out


def _build_nc(N):
    """Build the per-core graph for a compacted, padded sequence length N."""
    from contextlib import ExitStack

    import concourse.tile as tile
    from concourse import bacc, mybir
    from concourse.bass import ts, ds

    f32 = mybir.dt.float32
    bf16 = mybir.dt.bfloat16
    EXP = mybir.ActivationFunctionType.Exp

    ktiles = _chunks(N, P)  # [(koff, kh)] kh in {128, 64}
    nkt = len(ktiles)
    qchunks = _chunks(N, 512)

    nc = bacc.Bacc(None, target_bir_lowering=False, debug=False)

    xc_d = nc.declare_dram_parameter("xc", [D, N], bf16, isOutput=False)
    wq_d = nc.declare_dram_parameter("wq", [D, D], bf16, isOutput=False)
    wk_d = nc.declare_dram_parameter("wk", [D, D], bf16, isOutput=False)
    wv_d = nc.declare_dram_parameter("wv", [D, D], bf16, isOutput=False)
    bv_d = nc.declare_dram_parameter("bv", [1, D], f32, isOutput=False)
    kb_d = nc.declare_dram_parameter("kbias", [P, nkt], f32, isOutput=False)
    out_d = nc.declare_dram_parameter("out", [N, D], f32, isOutput=True)

    with tile.TileContext(nc) as tc, ExitStack() as st:
        const = st.enter_context(tc.tile_pool(name="const", bufs=1))
        persist = st.enter_context(tc.tile_pool(name="persist", bufs=1))
        # one PSUM ring shared by every stage — no pool-boundary barriers
        ps = st.enter_context(tc.tile_pool(name="ps", bufs=8, space="PSUM"))

        def psum(name, h, w):
            t = ps.tile([P, 512], f32, name=name, tag="ps")
            return t[:h, :w]

        ones_col = const.tile([P, 1], bf16, name="ones_col", tag="ones_col")
        nc.gpsimd.memset(ones_col, 1.0)
        kb_sb = const.tile([P, nkt], f32, name="kb_sb", tag="kb_sb")
        nc.sync.dma_start(kb_sb, kb_d[:, :])
        bv_sb = const.tile([1, D], f32, name="bv_sb", tag="bv_sb")
        nc.sync.dma_start(bv_sb, bv_d[:, :])
        bv_bc = const.tile([P, D], f32, name="bv_bc", tag="bv_bc")
        nc.gpsimd.partition_broadcast(bv_bc, bv_sb[:1, :])

        # PE pre-warm: dummy matmuls with no data deps run during the input
        # DMA lead-in so HAM un-throttles before the first real matmul.
        ws = const.tile([P, P], bf16, name="ws", tag="ws")
        nc.gpsimd.memset(ws, 0.0)
        for i in range(NWARM):
            wp = psum(f"wp{i}", P, P)
            nc.tensor.matmul(wp, lhsT=ws, rhs=ws, start=True, stop=True)

        qt = [
            persist.tile([P, N], bf16, name=f"qt{a}", tag="qt", bufs=NA)
            for a in range(NA)
        ]
        kt = [
            persist.tile([P, N], bf16, name=f"kt{a}", tag="kt", bufs=NA)
            for a in range(NA)
        ]
        v = [
            persist.tile([P, D], bf16, name=f"v{s}", tag="v", bufs=nkt)
            for s in range(nkt)
        ]

        # ---- phase 1: projections -------------------------------------
        with tc.tile_pool(name="xw", bufs=1) as xw:
            # interleave xc / wq loads so the q-projection is fed first
            xts, wq_t = [], []
            for it in range(NI):
                t = xw.tile([P, N], bf16, name=f"xtile{it}", tag="xt", bufs=NI)
                nc.sync.dma_start(t, xc_d[ts(it, P), :])
                xts.append(t)
                w = xw.tile([P, D], bf16, name=f"wqt{it}", tag="w", bufs=24)
                nc.sync.dma_start(w, wq_d[ts(it, P), :])
                wq_t.append(w)

            def load_w(dram, label):
                tiles = []
                for it in range(NI):
                    t = xw.tile([P, D], bf16, name=f"{label}{it}", tag="w", bufs=24)
                    nc.sync.dma_start(t, dram[ts(it, P), :])
                    tiles.append(t)
                return tiles

            wk_t = load_w(wk_d, "wkt")
            wv_t = load_w(wv_d, "wvt")

            # qT / kT: out[a-tile, chunk] = sum_i W.T[i, a-tile].T @ xcT[i, chunk]
            for wt, dst, lbl in ((wq_t, qt, "q"), (wk_t, kt, "k")):
                for a in range(NA):
                    pss = [
                        psum(f"pp_{lbl}{a}_{ci}", P, w)
                        for ci, (off, w) in enumerate(qchunks)
                    ]
                    for it in range(NI):
                        for ci, (off, w) in enumerate(qchunks):
                            nc.tensor.matmul(
                                pss[ci],
                                lhsT=wt[it][:, ts(a, P)],
                                rhs=xts[it][:, ds(off, w)],
                                start=(it == 0),
                                stop=(it == NI - 1),
                            )
                    for ci, (off, w) in enumerate(qchunks):
                        nc.vector.tensor_copy(dst[a][:, ds(off, w)], pss[ci])

            # v: out[k-tile, chunk] = sum_i xcT[i, k-tile].T @ Wv.T[i, chunk] + bv
            for s16, (koff, kh) in enumerate(ktiles):
                pss = [psum(f"pp_v{s16}_{c}", kh, VC) for c in range(NVC)]
                for it in range(NI):
                    for c in range(NVC):
                        nc.tensor.matmul(
                            pss[c],
                            lhsT=xts[it][:, ds(koff, kh)],
                            rhs=wv_t[it][:, ts(c, VC)],
                            start=(it == 0),
                            stop=(it == NI - 1),
                        )
                for c in range(NVC):
                    nc.vector.tensor_add(
                        v[s16][:kh, ts(c, VC)], pss[c], bv_bc[:kh, ts(c, VC)]
                    )

        # ---- phase 2: attention ---------------------------------------
        with (
            tc.tile_pool(name="etp", bufs=1) as etp,
            tc.tile_pool(name="outp", bufs=4) as outp,
            tc.tile_pool(name="smol", bufs=8) as smol,
        ):
            for qc, (qoff, qw) in enumerate(qchunks):
                ets = []
                for k16, (koff, kh) in enumerate(ktiles):
                    zps = psum(f"z{qc}_{k16}", kh, qw)
                    for a in range(NA):
                        nc.tensor.matmul(
                            zps,
                            lhsT=kt[a][:, ds(koff, kh)],
                            rhs=qt[a][:, ds(qoff, qw)],
                            start=(a == 0),
                            stop=(a == NA - 1),
                        )
                    et = etp.tile(
                        [P, 512],
                        bf16,
                        name=f"et{qc}_{k16}",
                        tag="et",
                        bufs=2 * nkt,
                    )[:kh, :qw]
                    nc.scalar.activation(
                        et, zps, EXP, bias=kb_sb[:kh, k16 : k16 + 1], scale=SCALE
                    )
                    ets.append(et)

                for qsoff, qh in _chunks(qw, P):
                    qrow = qoff + qsoff  # global compacted row
                    opss = [psum(f"pv{qrow}_{c}", qh, VC) for c in range(NVC)]
                    dps = psum(f"dn{qrow}", qh, 1)
                    for k16, (koff, kh) in enumerate(ktiles):
                        lhs = ets[k16][:, ds(qsoff, qh)]
                        for c in range(NVC):
                            nc.tensor.matmul(
                                opss[c],
                                lhsT=lhs,
                                rhs=v[k16][:kh, ts(c, VC)],
                                start=(k16 == 0),
                                stop=(k16 == nkt - 1),
                            )
                        nc.tensor.matmul(
                            dps,
                            lhsT=lhs,
                            rhs=ones_col[:kh, :1],
                            start=(k16 == 0),
                            stop=(k16 == nkt - 1),
                        )
                    rec = smol.tile([P, 1], f32, name=f"rec{qrow}", tag="rec")[:qh]
                    nc.vector.reciprocal(rec, dps)
                    for c in range(NVC):
                        ot = outp.tile([P, VC], f32, name=f"ot{qrow}_{c}", tag="ot")[
                            :qh
                        ]
                        nc.vector.tensor_scalar_mul(ot, opss[c], rec)
                        # split the store across partition stripes so it fans
                        # out over multiple DMA queues (a single queue is only
                        # ~22 GB/s; the last store otherwise dominates the tail)
                        for soff, sh in _chunks(qh, OSPLIT):
                            nc.sync.dma_start(
                                out_d[ds(qrow + soff, sh), ts(c, VC)],
                                ot[ds(soff, sh), :],
                            )

    nc.compile()
    return nc


def _get_nc(N):
    if N not in _CACHE:
        _CACHE[N] = _build_nc(N)
    return _CACHE[N]


def _make_in_maps(x, Wq, Wk, Wv, bv, mask, idxs, N):
    bf16 = ml_dtypes.bfloat16
    ktiles = _chunks(N, P)
    nkt = len(ktiles)
    wq_t = np.ascontiguousarray(Wq.astype(np.float32).T).astype(bf16)
    wk_t = np.ascontiguousarray(Wk.astype(np.float32).T).astype(bf16)
    wv_t = np.ascontiguousarray(Wv.astype(np.float32).T).astype(bf16)
    bv_row = np.ascontiguousarray(bv.astype(np.float32).reshape(1, D))
    in_maps = []
    for z in range(8):
        idx = idxs[z]
        n = idx.size
        idx_pad = np.zeros(N, dtype=np.int64)
        idx_pad[:n] = idx
        xc = np.ascontiguousarray(x[z][idx_pad].astype(np.float32).T).astype(bf16)
        kb = np.full(N, -30000.0, dtype=np.float32)
        kb[:n] = 0.0
        # kbias SBUF layout: column j covers compacted rows koff_j..koff_j+kh_j
        kbm = np.full((P, nkt), -30000.0, dtype=np.float32)
        for j, (koff, kh) in enumerate(ktiles):
            kbm[:kh, j] = kb[koff : koff + kh]
        in_maps.append(
            {
                "xc": xc,
                "wq": wq_t,
                "wk": wk_t,
                "wv": wv_t,
                "bv": bv_row,
                "kbias": np.ascontiguousarray(kbm),
            }
        )
    return in_maps


def run(x, Wq, Wk, Wv, bv, mask, trace=False):
    from concourse.bass_utils import run_bass_kernel_spmd

    x = np.asarray(x)
    mask = np.asarray(mask).astype(bool)
    idxs = [np.nonzero(~mask[z])[0] for z in range(8)]
    nmax = max(int(i.size) for i in idxs)
    N = max(GRAN, -(-nmax // GRAN) * GRAN)  # shared padded length
    nc = _get_nc(N)
    in_maps = _make_in_maps(x, Wq, Wk, Wv, bv, mask, idxs, N)
    res = run_bass_kernel_spmd(nc, in_maps, core_ids=list(range(8)), trace=trace)
    out = np.zeros((8, S, D), dtype=np.float32)
    for z in range(8):
        n = idxs[z].size
        if n:
            out[z][idxs[z]] = res.results[z]["out"][:n].astype(np.float32)
    return out, res


def kernel(x, Wq, Wk, Wv, bv, mask):
    out, _ = run(x, Wq, Wk, Wv, bv, mask, trace=False)
    return out
